# revision 1
# baseline (speedup 1.0000x reference)
"""LSTM-cell scan kernel for Trainium2 (8 NeuronCores, data-parallel over batch).

Problem: T=512 sequential LSTMCell steps, B=4096, I=10, H=20 (gates G=80).
Sharding: batch 4096 -> 8 cores x 512. Weights replicated. No cross-core comm.

Per-core design (B=512 local, 4 batch blocks of 128 on partitions):
  - State layout: batch on partitions. c: [128, (4,20)] f32. gates psum: [128, (4,80)].
  - Per step, 4 row-group-tiled matmuls (K=31, one per batch block a at partition
    base 32a) compute all gates: stationary = xh_aug^T slice [31,128] (bf16),
    moving = replicated W_aug^T [31,80] (bf16).  xh_aug rows per block:
    {0:10}=x^T, {10}=ones (bias row, appended to x on host), {11:31}=h^T.
  - The xh_aug^T stationary for step t+1 is produced by ONE PE transpose of a
    padded [128,128] bf16 tile (Hb) holding x_aug(t+1) (cast from DMA'd f32)
    and h(t) -- then one PSUM->SBUF copy.
  - Activations: one Sigmoid over all 320 gate cols (g-rows of W pre-scaled by 2
    on host, so tanh(g) = 2*sigmoid(2g)-1), plus one Tanh for c.
  - c update fused via scalar_tensor_tensor: w=(sg-0.5)*i; c=(2w)+f*c.
"""

import os
import sys

import numpy as np

sys.path.insert(0, "/opt/trn_rl_repo")

T, BFULL, I, H = 512, 4096, 10, 20
NCORES = 8
B = BFULL // NCORES  # 512 per core
G = 4 * H  # 80
IA = I + 2  # x augmented with ones column (bias row) + zero pad col
KAUG = IA + H  # 32
NBLK = 4  # batch blocks of 128 per core
LOOKAHEAD = 6
RS = LOOKAHEAD + 2  # x stage bufs
RH = 3  # Hb bufs
RT = 3  # slotT bufs

_compiled = None


_MAXW = 1  # max sem waits this walrus accepts attached to one instruction


def _split_waits(nc):
    """Hoist attached sem waits into standalone EventSemaphore instructions.

    This walrus build rejects instructions carrying more than one sync wait
    ("Too many sync wait commands").  For any instruction with multiple
    waits, emit standalone single-wait EventSemaphore instructions directly
    before it in the same engine stream (equivalent semantics: the engine
    queue stalls on each in turn).
    """
    from concourse import mybir

    for bbb in nc.bb_map.values():
        inner = bbb.bb
        insts = list(inner.instructions)
        out = []
        changed = False
        for inst in insts:
            si = getattr(inst, "sync_info", None)
            if si is not None and si.on_wait and len(si.on_wait) > _MAXW:
                waits = list(si.on_wait)
                for w in waits[:-_MAXW]:
                    ev = mybir.InstEventSemaphore(
                        name=nc.get_next_instruction_name(),
                        ins=[],
                        outs=[],
                    )
                    ev.engine = inst.engine
                    ev.sync_info = mybir.SyncInfo(on_wait=[w], on_update=[])
                    nc.register_instruction(ev)
                    out.append(ev)
                inst.sync_info = mybir.SyncInfo(
                    on_wait=waits[-_MAXW:], on_update=list(si.on_update or [])
                )
                changed = True
            out.append(inst)
        if changed:
            inner.instructions = out


def _build_bass(nsteps=T, variant=""):
    import concourse.bass as bass
    import concourse.tile as tile
    from concourse import mybir
    from concourse.masks import make_identity


    f32 = mybir.dt.float32
    bf16 = mybir.dt.bfloat16
    AF = mybir.ActivationFunctionType
    OP = mybir.AluOpType

    T_ = nsteps
    nc = bass.Bass()

    x_d = nc.dram_tensor("x", [T_, B, IA], f32, kind="ExternalInput")
    hx_d = nc.dram_tensor("hx", [B, H], f32, kind="ExternalInput")
    cx_d = nc.dram_tensor("cx", [B, H], f32, kind="ExternalInput")
    w_d = nc.dram_tensor("wT4", [128, G], f32, kind="ExternalInput")
    hs_d = nc.dram_tensor("hs", [T_, B, H], f32, kind="ExternalOutput")

    with tile.TileContext(nc) as tc:
        with (
            tc.tile_pool(name="const", bufs=1) as const,
            tc.tile_pool(name="stage", bufs=RS) as stagep,
            tc.tile_pool(name="hb", bufs=RH) as hbp,
            tc.tile_pool(name="slot", bufs=RT) as slotp,
            tc.tile_pool(name="sg", bufs=3) as sgp,
            tc.tile_pool(name="fc", bufs=3) as fcp,
            tc.tile_pool(name="wp", bufs=3) as wp,
            tc.tile_pool(name="th", bufs=3) as thp,
            tc.tile_pool(name="h32", bufs=4) as h32p,
            tc.tile_pool(name="psg", bufs=1, space="PSUM") as psg,
            tc.tile_pool(name="pst", bufs=2, space="PSUM") as pst,
        ):
            # ---- constants ----
            ident = const.tile([128, 128], bf16)
            make_identity(nc, ident)
            wf = const.tile([128, G], f32)
            nc.sync.dma_start(out=wf, in_=w_d[:, :])
            wb = const.tile([128, G], bf16)
            nc.vector.tensor_copy(wb, wf)

            # persistent cell state [128, 4, 20] f32
            C = const.tile([128, NBLK, H], f32)
            nc.sync.dma_start(
                out=C, in_=cx_d[:, :].rearrange("(a p) h -> p a h", p=128)
            )
            h0 = const.tile([128, NBLK, H], f32)
            nc.sync.dma_start(
                out=h0, in_=hx_d[:, :].rearrange("(a p) h -> p a h", p=128)
            )

            # Hb column structure within each 32-block a:
            #   cols 32a+{0:12}=x_aug (x, ones, zero), 32a+{12:32}=h
            def hb_x_view(hb):
                return hb.rearrange("p (a q) -> p a q", a=4)[:, :, 0:IA]

            def hb_h_view(hb):
                return hb.rearrange("p (a q) -> p a q", a=4)[:, :, IA:KAUG]

            # ---- prologue ----
            stages = []
            for k in range(min(LOOKAHEAD, T_)):
                st = stagep.tile([128, NBLK, IA], f32, tag="stage")
                nc.gpsimd.dma_start(
                    out=st, in_=x_d[k].rearrange("(a p) i -> p a i", p=128)
                )
                stages.append(st)

            hb0 = hbp.tile([128, 128], bf16, tag="hb")
            nc.vector.tensor_copy(hb_x_view(hb0), stages[0])
            nc.vector.tensor_copy(hb_h_view(hb0), h0)
            pT = pst.tile([128, 128], bf16, tag="pst")
            nc.tensor.transpose(pT, hb0, ident)
            slot = slotp.tile([128, 128], bf16, tag="slot")
            nc.vector.tensor_copy(slot, pT)

            # ---- main loop, fully unrolled ----
            for t in range(T_):
                # gates matmuls: 4 row-group-tiled blocks
                # one PSUM bank per block: concurrent row-group matmuls
                # must not drain into the same bank
                pg = psg.tile([128, NBLK, 512], f32, tag="psg")
                nmm = 0 if "nomm" in variant else (1 if "onemm" in variant else (2 if "mm2" in variant else (3 if "mm3" in variant else NBLK)))
                for a in range(nmm):
                    nc.tensor.matmul(
                        pg[:, a, 0:G],
                        lhsT=slot[32 * a : 32 * a + KAUG, :],
                        rhs=wb[32 * a : 32 * a + KAUG, :],
                        start=True,
                        stop=True,
                        tile_position=(32 * a, 0),
                    )
                if nmm == 0:
                    nc.vector.memset(pg[:, :, 0:G], 0.0)
                # sigmoid over all gates (g pre-scaled by 2 -> tanh fix later)
                S = sgp.tile([128, NBLK * G], bf16, tag="sg")
                nc.scalar.activation(S, pg[:, :, 0:G], AF.Sigmoid)
                Sv = S.rearrange("p (a g) -> p a g", a=NBLK)
                Si = Sv[:, :, 0:20]
                Sf = Sv[:, :, 20:40]
                Sg = Sv[:, :, 40:60]
                So = Sv[:, :, 60:80]

                # c update: fc = f*c ; w = (sg-0.5)*i ; c = 2w + fc
                FC = fcp.tile([128, NBLK, H], f32, tag="fc")
                nc.vector.tensor_mul(FC, Sf, C)
                W = wp.tile([128, NBLK, H], bf16, tag="wp")
                nc.vector.scalar_tensor_tensor(
                    W, in0=Sg, scalar=0.5, in1=Si, op0=OP.subtract, op1=OP.mult
                )
                nc.vector.scalar_tensor_tensor(
                    C, in0=W, scalar=2.0, in1=FC, op0=OP.mult, op1=OP.add
                )
                TH = thp.tile([128, NBLK, H], bf16, tag="th")
                nc.scalar.activation(TH, C, AF.Tanh)

                # h (f32) -> DRAM
                H32 = h32p.tile([128, NBLK, H], f32, tag="h32")
                nc.vector.tensor_mul(H32, So, TH)
                nc.sync.dma_start(
                    out=hs_d[t].rearrange("(a p) h -> p a h", p=128), in_=H32
                )

                if t + LOOKAHEAD < T_:
                    st = stagep.tile([128, NBLK, IA], f32, tag="stage")
                    nc.gpsimd.dma_start(
                        out=st,
                        in_=x_d[t + LOOKAHEAD].rearrange("(a p) i -> p a i", p=128),
                    )
                    stages.append(st)

                if t < T_ - 1 and "notrans" not in variant:
                    # build Hb(t+1): x_aug(t+1) cast f32->bf16, h(t) bf16
                    hb = hbp.tile([128, 128], bf16, tag="hb")
                    nc.vector.tensor_copy(hb_x_view(hb), stages[t + 1])
                    nc.vector.tensor_mul(hb_h_view(hb), So, TH)
                    pT = pst.tile([128, 128], bf16, tag="pst")
                    nc.tensor.transpose(pT, hb, ident)
                    slot = slotp.tile([128, 128], bf16, tag="slot")
                    nc.vector.tensor_copy(slot, pT)

    _split_waits(nc)
    return nc


def _get_compiled():
    global _compiled
    if _compiled is None:
        _compiled = _build_bass()
    return _compiled


def _prep_w(W_ih, W_hh, b_ih, b_hh):
    # augmented weight [G, KAUG]: cols 0:10 = W_ih, col 10 = bias,
    # col 11 = zero pad, cols 12:32 = W_hh
    Waug = np.zeros((G, KAUG), dtype=np.float32)
    Waug[:, 0:I] = W_ih
    Waug[:, I] = b_ih + b_hh
    Waug[:, IA:] = W_hh
    Waug[40:60, :] *= 2.0  # g rows: sigmoid(2g) trick
    wT4 = np.zeros((128, G), dtype=np.float32)
    for a in range(NBLK):
        wT4[32 * a : 32 * a + KAUG, :] = Waug.T
    return wT4


def _prep_x(x):
    # append ones column: [T, B, I] -> [T, B, I+1]
    tdim, bdim, _ = x.shape
    xa = np.empty((tdim, bdim, IA), dtype=np.float32)
    xa[:, :, :I] = x
    xa[:, :, I] = 1.0
    xa[:, :, I + 1] = 0.0
    return xa


def kernel(x, hx, cx, W_ih, W_hh, b_ih, b_hh):
    from concourse.bass_utils import run_bass_kernel_spmd

    nc = _get_compiled()
    xa = _prep_x(np.asarray(x, dtype=np.float32))
    hx = np.asarray(hx, dtype=np.float32)
    cx = np.asarray(cx, dtype=np.float32)
    wT4 = _prep_w(
        np.asarray(W_ih, np.float32),
        np.asarray(W_hh, np.float32),
        np.asarray(b_ih, np.float32),
        np.asarray(b_hh, np.float32),
    )

    in_maps = []
    for k in range(NCORES):
        sl = slice(k * B, (k + 1) * B)
        in_maps.append(
            {
                "x": np.ascontiguousarray(xa[:, sl, :]),
                "hx": np.ascontiguousarray(hx[sl]),
                "cx": np.ascontiguousarray(cx[sl]),
                "wT4": wT4,
            }
        )

    res = run_bass_kernel_spmd(nc, in_maps, list(range(NCORES)))
    outs = [res.results[k]["hs"] for k in range(NCORES)]
    return np.concatenate(outs, axis=1).astype(np.float32)



# revision 18
# speedup vs baseline: 1.1197x; 1.1197x over previous
"""LSTM-cell scan kernel for Trainium2 (8 NeuronCores, data-parallel over batch).

Problem: T=512 sequential LSTMCell steps, B=4096, I=10, H=20 (gates G=80).
Sharding: batch 4096 -> 8 cores x 512. Weights replicated. No cross-core comm.

v2 design: two staggered half-batch chains per core (2 x 256 batch) so the
irreducible per-step recurrence latency of one chain hides under the other's
engine work.  All of x lives in SBUF (loaded once, bf16), embedded in a giant
per-half "hb" tensor [128, T+1, 64] whose sub-slot t holds [x_aug(t) | h(t-1)]
per 32-col block; h is written in place each step, so there are ZERO per-step
DMAs.  Output h is DMA'd out once at the end (bf16; host upcasts).

Per half-chain step (blocks b=0,1 of 128 batch on partitions):
  PE:   transpose hb[:,t,:] -> pT [64,128]; 2 row-grouped matmuls
        (stationary = slot rows 32b, moving = replicated W_aug^T) -> psum gates
  Act:  Sigmoid over [128,2,80] (g-rows pre-scaled by 2: tanh(g)=2*sig(2g)-1),
        Tanh over c [128,2,20]
  DVE:  W=(Sg-.5)*Si ; C=2W+FC ; h=So*TH into hb h-cols ; slot copy psum->sbuf
  Pool: FC=Sf*C
"""

import os
import sys

import numpy as np

sys.path.insert(0, "/opt/trn_rl_repo")

T, BFULL, I, H = 512, 4096, 10, 20
NCORES = 8
B = BFULL // NCORES  # 512 per core
G = 4 * H  # 80
IA = I + 2  # x augmented with ones column (bias row) + zero pad col
KAUG = IA + H  # 32
NHALF = 2  # chains per core
NBLK = 2  # batch blocks of 128 per chain

_compiled = None

_MAXW = 1  # max sem waits this walrus accepts attached to one instruction


def _split_waits(nc):
    """Hoist attached sem waits into standalone EventSemaphore instructions.

    This walrus build rejects instructions carrying more than one sync wait
    ("Too many sync wait commands").  For any instruction with multiple
    waits, emit standalone single-wait EventSemaphore instructions directly
    before it in the same engine stream (equivalent semantics: the engine
    queue stalls on each in turn).
    """
    from concourse import mybir

    for bbb in nc.bb_map.values():
        inner = bbb.bb
        insts = list(inner.instructions)
        out = []
        changed = False
        for inst in insts:
            si = getattr(inst, "sync_info", None)
            if si is not None and si.on_wait and len(si.on_wait) > _MAXW:
                waits = list(si.on_wait)
                for w in waits[:-_MAXW]:
                    ev = mybir.InstEventSemaphore(
                        name=nc.get_next_instruction_name(),
                        ins=[],
                        outs=[],
                    )
                    ev.engine = inst.engine
                    ev.sync_info = mybir.SyncInfo(on_wait=[w], on_update=[])
                    nc.register_instruction(ev)
                    out.append(ev)
                inst.sync_info = mybir.SyncInfo(
                    on_wait=waits[-_MAXW:], on_update=list(si.on_update or [])
                )
                changed = True
            out.append(inst)
        if changed:
            inner.instructions = out


def _build_bass(nsteps=T, variant=""):
    import concourse.bass as bass
    import concourse.tile as tile
    from concourse import mybir
    from concourse.masks import make_identity

    f32 = mybir.dt.float32
    bf16 = mybir.dt.bfloat16
    AF = mybir.ActivationFunctionType
    OP = mybir.AluOpType

    T_ = nsteps
    nc = bass.Bass()

    # DRAM tensors, all pre-packed on host:
    #  x{m}:  [128, T+1, 64] bf16 — the FULL hb image, partition-major:
    #         sub-slot t cols 32b+{0:12} = x_aug(t), sub-slot 0 cols
    #         32b+{12:32} = hx, everything else 0.  One fat contiguous DMA.
    #  cx{m}: [128, 2, 20] f32
    #  wT4:   [128, G] bf16  (W_aug^T replicated at 32-row offsets, g rows x2)
    #  hs{m}: [128, T, 64] bf16 out — raw dump of sub-slots 1..T (h at cols
    #         32b+{12:32}; host slices).
    x_d = [
        nc.dram_tensor(
            f"x{m}", [128, T_ + 1, NBLK * KAUG], bf16, kind="ExternalInput"
        )
        for m in range(NHALF)
    ]
    cx_d = [
        nc.dram_tensor(f"cx{m}", [128, NBLK, H], f32, kind="ExternalInput")
        for m in range(NHALF)
    ]
    w_d = nc.dram_tensor("wT4", [128, G], bf16, kind="ExternalInput")
    hs_d = [
        nc.dram_tensor(
            f"hs{m}", [128, T_, NBLK * KAUG], bf16, kind="ExternalOutput"
        )
        for m in range(NHALF)
    ]

    fc_engine = "gpsimd" if "fcgps" in variant else "vector"
    copy_engine = "gpsimd" if "cpgps" in variant else "vector"

    with tile.TileContext(nc) as tc:
        with (
            tc.tile_pool(name="const", bufs=1) as const,
            tc.tile_pool(name="slotA", bufs=2) as slotpA,
            tc.tile_pool(name="slotB", bufs=2) as slotpB,
            tc.tile_pool(name="sgA", bufs=2) as sgpA,
            tc.tile_pool(name="sgB", bufs=2) as sgpB,
            tc.tile_pool(name="fcA", bufs=2) as fcpA,
            tc.tile_pool(name="fcB", bufs=2) as fcpB,
            tc.tile_pool(name="wpA", bufs=2) as wpA,
            tc.tile_pool(name="wpB", bufs=2) as wpB,
            tc.tile_pool(name="thA", bufs=2) as thpA,
            tc.tile_pool(name="thB", bufs=2) as thpB,
            tc.tile_pool(name="psgA", bufs=1, space="PSUM") as psgA,
            tc.tile_pool(name="psgB", bufs=1, space="PSUM") as psgB,
            tc.tile_pool(name="pstA", bufs=1, space="PSUM") as pstA,
            tc.tile_pool(name="pstB", bufs=1, space="PSUM") as pstB,
        ):
            slotp = [slotpA, slotpB]
            sgp = [sgpA, sgpB]
            fcp = [fcpA, fcpB]
            wp = [wpA, wpB]
            thp = [thpA, thpB]
            psg = [psgA, psgB]
            pst = [pstA, pstB]

            # ---- constants ----
            ident = const.tile([128, 128], bf16)
            make_identity(nc, ident)
            wb = const.tile([128, G], bf16)
            nc.sync.dma_start(out=wb, in_=w_d[:, :])

            # persistent cell state per half [128, 2, 20] f32
            C = []
            for m in range(NHALF):
                cm = const.tile([128, NBLK, H], f32, name=f"C{m}")
                nc.sync.dma_start(out=cm, in_=cx_d[m][:, :, :])
                C.append(cm)

            # giant hb per half: [128, T+1, 64] bf16.
            # sub-slot t cols (b,q): q 0:12 = x_aug(t), q 12:32 = h(t-1).
            HB = []
            for m in range(NHALF):
                hbm = const.tile([128, T_ + 1, NBLK * KAUG], bf16, name=f"HB{m}")
                HB.append(hbm)

            def hb_slot(m, t):
                return HB[m][:, t, :]  # [128, 64]



            def hb_h(m, t):
                # h-cols of sub-slot t: [128, 2, 20]
                return HB[m].rearrange("p t (b q) -> p t b q", b=NBLK)[
                    :, t, :, IA:KAUG
                ]

            # ---- prologue: load the full hb image (x + hx + zeros) ----
            # chunked along T: a single DMA's element-count field is 16-bit
            xchunk = 128
            for m in range(NHALF):
                for t0 in range(0, T_ + 1, xchunk):
                    t1 = min(t0 + xchunk, T_ + 1)
                    nc.sync.dma_start(
                        out=HB[m][:, t0:t1, :], in_=x_d[m][:, t0:t1, :]
                    )

            # half m's transposed slot lives at partitions 64m:64m+64 so the
            # gates matmul's lhsT partition start matches its wb slice
            # (walrus: "Fmap and Weight must start at the same partition").
            def make_slot(m, t):
                pT = pst[m].tile([128, 128], bf16, tag=f"pst{m}")
                nc.tensor.transpose(
                    pT[64 * m : 64 * m + 64, :],
                    hb_slot(m, t),
                    ident,
                    tile_position=(0, 64 * m),
                )
                sl = slotp[m].tile([128, 128], bf16, tag=f"slot{m}")
                getattr(nc, copy_engine).tensor_copy(
                    sl[64 * m : 64 * m + 64, :], pT[64 * m : 64 * m + 64, :]
                )
                return sl

            slot = [make_slot(m, 0) for m in range(NHALF)]

            # ---- main loop, fully unrolled, halves interleaved ----
            for t in range(T_):
                for m in range(NHALF):
                    # gates matmuls: 2 row-grouped blocks -> 2 psum banks
                    pg = psg[m].tile([128, NBLK, 512], f32, tag=f"psg{m}")
                    for b in range(NBLK):
                        r = 64 * m + 32 * b
                        nc.tensor.matmul(
                            pg[:, b, 0:G],
                            lhsT=slot[m][r : r + KAUG, :],
                            rhs=wb[r : r + KAUG, :],
                            start=True,
                            stop=True,
                            tile_position=(r, 0),
                        )
                    # sigmoid over all gates (g rows pre-scaled by 2)
                    S = sgp[m].tile([128, NBLK, G], bf16, tag=f"sg{m}")
                    nc.scalar.activation(S, pg[:, :, 0:G], AF.Sigmoid)
                    Si = S[:, :, 0:20]
                    Sf = S[:, :, 20:40]
                    Sg = S[:, :, 40:60]
                    So = S[:, :, 60:80]

                    # c update: fc = f*c ; w = (sg-0.5)*i ; c = 2w + fc
                    FC = fcp[m].tile([128, NBLK, H], f32, tag=f"fc{m}")
                    getattr(nc, fc_engine).tensor_mul(FC, Sf, C[m])
                    W = wp[m].tile([128, NBLK, H], bf16, tag=f"wp{m}")
                    nc.vector.scalar_tensor_tensor(
                        W, in0=Sg, scalar=0.5, in1=Si, op0=OP.subtract, op1=OP.mult
                    )
                    nc.vector.scalar_tensor_tensor(
                        C[m], in0=W, scalar=2.0, in1=FC, op0=OP.mult, op1=OP.add
                    )
                    TH = thp[m].tile([128, NBLK, H], bf16, tag=f"th{m}")
                    nc.scalar.activation(TH, C[m], AF.Tanh)

                    # h(t) = o * tanh(c) -> hb sub-slot t+1 h-cols (also = hs[t])
                    nc.vector.tensor_mul(hb_h(m, t + 1), So, TH)

                    if t + 1 < T_:
                        slot[m] = make_slot(m, t + 1)

            # ---- epilogue: raw dump of sub-slots 1..T, chunked ----
            for m in range(NHALF):
                for t0 in range(0, T_, xchunk):
                    t1 = min(t0 + xchunk, T_)
                    nc.sync.dma_start(
                        out=hs_d[m][:, t0:t1, :],
                        in_=HB[m][:, t0 + 1 : t1 + 1, :],
                    )

    _split_waits(nc)
    return nc


def _get_compiled():
    global _compiled
    if _compiled is None:
        _compiled = _build_bass()
    return _compiled


def _prep_w(W_ih, W_hh, b_ih, b_hh):
    import ml_dtypes

    # augmented weight [G, KAUG]: cols 0:10 = W_ih, col 10 = bias,
    # col 11 = zero pad, cols 12:32 = W_hh
    Waug = np.zeros((G, KAUG), dtype=np.float32)
    Waug[:, 0:I] = W_ih
    Waug[:, I] = b_ih + b_hh
    Waug[:, IA:] = W_hh
    Waug[40:60, :] *= 2.0  # g rows: sigmoid(2g) trick
    wT4 = np.zeros((128, G), dtype=np.float32)
    for a in range(4):
        wT4[32 * a : 32 * a + KAUG, :] = Waug.T
    return wT4.astype(ml_dtypes.bfloat16)


def build_in_maps(x, hx, cx, W_ih, W_hh, b_ih, b_hh):
    """Host-side packing: per-core, per-half tensors as the kernel expects."""
    import ml_dtypes

    bf16 = ml_dtypes.bfloat16
    x = np.asarray(x, np.float32)
    hx = np.asarray(hx, np.float32)
    cx = np.asarray(cx, np.float32)
    wT4 = _prep_w(
        np.asarray(W_ih, np.float32),
        np.asarray(W_hh, np.float32),
        np.asarray(b_ih, np.float32),
        np.asarray(b_hh, np.float32),
    )

    # Full hb image: [core, half, 128(p), T+1, 2(b), 32]
    # sub-slot t cols 32b+{0:12} = x_aug(t) (t<T); sub-slot 0 cols
    # 32b+{12:32} = hx; else 0.  batch = core*512 + half*256 + block*128 + p
    img = np.zeros((NCORES, NHALF, 128, T + 1, NBLK, KAUG), dtype=np.float32)
    # x: [T, 4096, 10] -> [core, half, p, t, b, i]
    x6 = x.reshape(T, NCORES, NHALF, NBLK, 128, I).transpose(1, 2, 4, 0, 3, 5)
    img[:, :, :, :T, :, 0:I] = x6
    img[:, :, :, :T, :, I] = 1.0
    h6 = hx.reshape(NCORES, NHALF, NBLK, 128, H).transpose(0, 1, 3, 2, 4)
    img[:, :, :, 0, :, IA:KAUG] = h6
    img = np.ascontiguousarray(img).astype(bf16)

    c6 = cx.reshape(NCORES, NHALF, NBLK, 128, H).transpose(0, 1, 3, 2, 4)
    c6 = np.ascontiguousarray(c6).astype(np.float32)

    in_maps = []
    for k in range(NCORES):
        im = {"wT4": wT4}
        for m in range(NHALF):
            im[f"x{m}"] = img[k, m].reshape(128, T + 1, NBLK * KAUG)
            im[f"cx{m}"] = c6[k, m]
        in_maps.append(im)
    return in_maps


def unshard_output(results):
    """results: list per core of {hs0, hs1: [128,T,64] bf16} -> [T,4096,20] f32."""
    outs = np.stack(
        [
            np.stack([np.asarray(results[k][f"hs{m}"]) for m in range(NHALF)])
            for k in range(NCORES)
        ]
    ).reshape(NCORES, NHALF, 128, T, NBLK, KAUG)[:, :, :, :, :, IA:KAUG]
    # [core, half, 128(p), T, 2(b), 20]
    outs = outs.astype(np.float32).transpose(3, 0, 1, 4, 2, 5)
    # -> [T, core, half, 2(b), 128(p), 20]
    return np.ascontiguousarray(outs.reshape(T, BFULL, H))


def kernel(x, hx, cx, W_ih, W_hh, b_ih, b_hh):
    from concourse.bass_utils import run_bass_kernel_spmd

    nc = _get_compiled()
    in_maps = build_in_maps(x, hx, cx, W_ih, W_hh, b_ih, b_hh)
    res = run_bass_kernel_spmd(nc, in_maps, list(range(NCORES)))
    return unshard_output(res.results)
